# revision 1
# baseline (speedup 1.0000x reference)
"""Multi-head causal attention (B=8, T=2048, C=384, H=6, Dh=64) on 8 TRN2 cores.

Sharding: data-parallel over batch — core b computes batch element b end to end
(no collectives).

Per-core kernel layout (all "T" means transposed, head-dim/channel on
partitions):
  xT   [128, 3, 2048]  bf16   c = 128*ci + p
  wq/wk[128, 3, 384]   bf16   packed Wq[h,c,d] -> [c, h*64+d]
  wv   [128, 3, 384]   bf16
  wp   [128, 3, 384]   bf16   Wp[c, e] -> [128, ci, e]
  mask [128, 384]      f32    mask[p, g] = 0 if p <= g-128 else -1e30
  bp   [1, 384]        f32r   bias row (K=1 matmul into output PSUM)

Compute per core:
  QT/KT [hd, t] via matmul(lhsT=w chunk, rhs=xT)      (hd = h*64+d, 3 blocks)
  V_aug [s, 65] per (s-chunk, head), last col = 1     (stationary for PV)
  per q-block j (256 wide), head h:
    ST chunks [s=128, t=256] = KT^T-slice @ QT-slice  (K = d = 64)
    causal mask add on diagonal chunk, exp (ACT, scale=Dh^-0.5) -> P bf16
    O_aug [65, 256] += V_aug^T @ P                    (row 64 = softmax denom)
    recip = 1/denom; B = ones64^T @ recip (K=1)       (broadcast over d)
    attT [hd, t] slice = O[0:64] * B                  (DVE, bf16)
  out [t, e] = attT^T-slice @ wp + ones128^T @ bp     (K = hd, 3 chunks + bias)
"""

import numpy as np
import ml_dtypes

import concourse.bass as bass
import concourse.tile as tile
from concourse import bacc, mybir
from concourse.bass import ts, ds

F32 = mybir.dt.float32
F32R = mybir.dt.float32r
BF16 = mybir.dt.bfloat16
AF = mybir.ActivationFunctionType

B, T, C = 8, 2048, 384
H, DH = 6, 64
SCALE = DH ** -0.5
NEG = -1e30
NCORES = 8
TJ = 512            # q-block width
NJ = T // TJ        # 8 q-blocks
SC = 128            # s-chunk
NCI = C // 128      # 3 channel chunks


def build_kernel():
    nc = bacc.Bacc("TRN2", target_bir_lowering=False, debug=False)

    xT_d = nc.dram_tensor("xT", [128, NCI, T], BF16, kind="ExternalInput").ap()
    wq_d = nc.dram_tensor("wq", [128, NCI, C], BF16, kind="ExternalInput").ap()
    wk_d = nc.dram_tensor("wk", [128, NCI, C], BF16, kind="ExternalInput").ap()
    wv_d = nc.dram_tensor("wv", [128, NCI, C], BF16, kind="ExternalInput").ap()
    wp_d = nc.dram_tensor("wp", [128, NCI, C], BF16, kind="ExternalInput").ap()
    mask_d = nc.dram_tensor("mask", [128, 128], F32, kind="ExternalInput").ap()
    biasb_d = nc.dram_tensor("biasb", [128, 384], F32, kind="ExternalInput").ap()
    iden_d = nc.dram_tensor("iden", [128, 128], F32, kind="ExternalInput").ap()
    y_d = nc.dram_tensor("y", [T, C], F32, kind="ExternalOutput").ap()

    with tile.TileContext(nc) as tc:
        with tc.tile_pool(name="const", bufs=1) as cpool:
            xT = cpool.tile([128, NCI, T], BF16)
            wq = cpool.tile([128, NCI, C], BF16)
            wk = cpool.tile([128, NCI, C], BF16)
            wv = cpool.tile([128, NCI, C], BF16)
            wp = cpool.tile([128, NCI, C], BF16)
            mask = cpool.tile([128, 128], F32)
            biasb = cpool.tile([128, 384], F32)
            iden = cpool.tile([128, 128], F32)
            QT = cpool.tile([128, NCI, T], BF16)
            KT = cpool.tile([128, NCI, T], BF16)
            attT = cpool.tile([128, NCI, T], BF16)
            Vt = cpool.tile([128, 16, H, 65], BF16)

            for ci in range(NCI):
                nc.sync.dma_start(xT[:, ci, :], xT_d[:, ci, :])
            nc.sync.dma_start(wq[:], wq_d[:])
            nc.sync.dma_start(wk[:], wk_d[:])
            nc.sync.dma_start(wv[:], wv_d[:])
            nc.sync.dma_start(wp[:], wp_d[:])
            nc.sync.dma_start(mask[:], mask_d[:])
            nc.sync.dma_start(biasb[:], biasb_d[:])
            nc.sync.dma_start(iden[:], iden_d[:])
            # whole-tile memset (contiguous; strided memset fails ISA check);
            # V copies below overwrite cols 0:64, leaving col 64 == 1.0
            nc.gpsimd.memset(Vt[:], 1.0)

            # ---- phase 1: projections ----
            with tc.tile_pool(name="pqk", bufs=2, space="PSUM") as pqk, \
                 tc.tile_pool(name="pv", bufs=2, space="PSUM") as pvp:
                for dst, w in ((QT, wq), (KT, wk)):
                    for pi in range(NCI):
                        for tcn in range(T // 512):
                            ps = pqk.tile([128, 512], F32, tag="pqk")
                            for ci in range(NCI):
                                nc.tensor.matmul(
                                    ps[:],
                                    lhsT=w[:, ci, ts(pi, 128)],
                                    rhs=xT[:, ci, ts(tcn, 512)],
                                    start=(ci == 0), stop=(ci == NCI - 1),
                                )
                            nc.vector.tensor_copy(dst[:, pi, ts(tcn, 512)], ps[:])
                for si in range(16):
                    ps = pvp.tile([128, C], F32, tag="pv")
                    for ci in range(NCI):
                        nc.tensor.matmul(
                            ps[:],
                            lhsT=xT[:, ci, ts(si, 128)],
                            rhs=wv[:, ci, :],
                            start=(ci == 0), stop=(ci == NCI - 1),
                        )
                    nc.vector.tensor_copy(
                        Vt[:, si, :, 0:64],
                        ps[:].rearrange("p (h d) -> p h d", h=H),
                    )

            # ---- phase 2+3: attention + output projection ----
            with tc.tile_pool(name="sps", bufs=2, space="PSUM") as sps, \
                 tc.tile_pool(name="ops", bufs=2, space="PSUM") as ops, \
                 tc.tile_pool(name="dps", bufs=2, space="PSUM") as dps, \
                 tc.tile_pool(name="ups", bufs=2, space="PSUM") as ups, \
                 tc.tile_pool(name="pp", bufs=4) as pp, \
                 tc.tile_pool(name="rp", bufs=2) as rp, \
                 tc.tile_pool(name="yp", bufs=2) as yp:
                NCH = TJ // SC  # s-chunks per q-block (4)
                for j in range(NJ):
                    # denominators of all 6 heads, transposed: dT[t%128, h*4+q]
                    dT = dps.tile([128, NCH * H], F32, tag="dT")
                    for h in range(H):
                        po = (h % 2) * 64     # partition offset inside hd-block
                        bi = h // 2           # hd block index
                        O = ops.tile([65, TJ], F32, tag="O")
                        for i in range(NCH * j + NCH):
                            fringe = i >= NCH * j
                            d = SC * i - TJ * j if fringe else 0
                            S = sps.tile([128, TJ], F32, tag="S")
                            nc.tensor.matmul(
                                S[:, d:TJ],
                                lhsT=KT[po:po + 64, bi, ts(i, SC)],
                                rhs=QT[po:po + 64, bi, ds(j * TJ + d, TJ - d)],
                                start=True, stop=True,
                            )
                            P = pp.tile([128, TJ], BF16, tag="P")
                            nc.scalar.activation(P[:, d:TJ], S[:, d:TJ],
                                                 AF.Exp, scale=SCALE)
                            if fringe:
                                if d > 0:
                                    nc.gpsimd.memset(P[:, 0:d], 0.0)
                                # diagonal window [d, d+128): keep iff p <= f-d
                                nc.gpsimd.affine_select(
                                    out=P[:, d:d + 128], in_=P[:, d:d + 128],
                                    pattern=[[1, 128]],
                                    compare_op=mybir.AluOpType.is_ge,
                                    fill=0.0, base=0, channel_multiplier=-1,
                                )
                            nc.tensor.matmul(
                                O[:],
                                lhsT=Vt[:, i, h, :],
                                rhs=P[:],
                                start=(i == 0), stop=(i == NCH * j + NCH - 1),
                            )
                        # stage unnormalized attT (bf16) and transposed denom
                        nc.vector.tensor_copy(
                            attT[po:po + 64, bi, ts(j, TJ)], O[0:64, :]
                        )
                        dsb = rp.tile([1, TJ], F32, tag="dsb")
                        nc.vector.tensor_copy(dsb[:], O[64:65, :])
                        for q in range(NCH):
                            nc.tensor.transpose(
                                dT[:, h * NCH + q:h * NCH + q + 1],
                                dsb[0:1, ts(q, 128)], iden[0:1, 0:1],
                            )
                    rT = rp.tile([128, NCH * H], F32, tag="rT")
                    nc.vector.reciprocal(rT[:], dT[:])
                    # ---- per-head output projection, normalized via stt ----
                    for q in range(NCH):
                        tb = NCH * j + q
                        Y = yp.tile([128, C], F32, tag="Y")
                        for h in range(H):
                            po = (h % 2) * 64
                            bi = h // 2
                            U = ups.tile([128, C], F32, tag="U")
                            nc.tensor.matmul(
                                U[:],
                                lhsT=attT[po:po + 64, bi, ts(tb, 128)],
                                rhs=wp[po:po + 64, bi, :],
                                start=True, stop=True,
                            )
                            sc = rT[:, h * NCH + q:h * NCH + q + 1]
                            nc.vector.scalar_tensor_tensor(
                                out=Y[:], in0=U[:], scalar=sc,
                                in1=(biasb[:] if h == 0 else Y[:]),
                                op0=mybir.AluOpType.mult,
                                op1=mybir.AluOpType.add,
                            )
                        nc.sync.dma_start(y_d[ts(tb, 128), :], Y[:])

    nc.compile()
    return nc


def _prep_inputs(x, Wq, Wk, Wv, Wp, bp):
    """Host-side shard + layout prep. Returns per-core input maps."""
    bf = ml_dtypes.bfloat16
    x = np.asarray(x, dtype=np.float32)

    def pack_w(W):  # [H, C, Dh] -> [128, NCI, H*Dh]
        Whd = np.transpose(np.asarray(W, np.float32), (1, 0, 2)).reshape(C, H * DH)
        return np.ascontiguousarray(
            Whd.reshape(NCI, 128, H * DH).transpose(1, 0, 2)
        ).astype(bf)

    wq_p, wk_p, wv_p = pack_w(Wq), pack_w(Wk), pack_w(Wv)
    wp_p = np.ascontiguousarray(
        np.asarray(Wp, np.float32).reshape(NCI, 128, C).transpose(1, 0, 2)
    ).astype(bf)

    f = np.arange(128)[None, :]
    p = np.arange(128)[:, None]
    mask = np.where(p <= f, 0.0, NEG).astype(np.float32)
    biasb = np.broadcast_to(np.asarray(bp, np.float32), (128, C)).copy()
    iden_np = np.eye(128, dtype=np.float32)

    in_maps = []
    for b in range(B):
        xT = np.ascontiguousarray(
            x[b].T.reshape(NCI, 128, T).transpose(1, 0, 2)
        ).astype(bf)
        in_maps.append({
            "xT": xT, "wq": wq_p, "wk": wk_p, "wv": wv_p, "wp": wp_p,
            "mask": mask, "biasb": biasb, "iden": iden_np,
        })
    return in_maps


_CACHE = {}


def kernel(x, Wq, Wk, Wv, Wp, bp):
    from concourse.bass_utils import run_bass_kernel_spmd

    if "nc" not in _CACHE:
        _CACHE["nc"] = build_kernel()
    nc = _CACHE["nc"]
    in_maps = _prep_inputs(x, Wq, Wk, Wv, Wp, bp)
    res = run_bass_kernel_spmd(nc, in_maps, list(range(NCORES)))
    out = np.stack([res.results[b]["y"] for b in range(B)], axis=0)
    return out.astype(np.float32)



# revision 4
# speedup vs baseline: 1.0939x; 1.0939x over previous
"""Multi-head causal attention (B=8, T=2048, C=384, H=6, Dh=64) on 8 TRN2 cores.

Sharding: data-parallel over batch — core b computes batch element b end to end
(no collectives).

v2 design notes (vs v1):
  - Head-PAIR processing: heads (2bi, 2bi+1) live on partition halves
    [0:64) / [64:128) of hd-block bi. Their S matmuls (K=64) are issued
    back-to-back so the PE runs them CONCURRENTLY via row tiling
    (tile_position auto-derived from base_partition 0 / 64).
  - One wide ACTIVATE (exp) per chunk-pair over S2 [128, 2, 512-d]
    (both heads at once) — halves ACT instruction overhead.
  - PV fringe-trimmed: matmul only cols [d:512] (P[:, :d] is never read
    → no memsets). Softmax denominator via V_aug ones-row (M=65).
  - Normalization: denominator rows broadcast across partitions on
    GpSimd (partition_broadcast), reciprocal on DVE, then one
    tensor_tensor multiply straight out of PSUM O → attT (bf16).
    No PE transposes, no per-head STT chains.
  - Output projection packed at K=128 (3 matmuls per 128-token block),
    bias added by the PSUM→SBUF scalar_tensor_tensor copy.
  - QKV projections are NOT a separate phase: proj for q-block j is
    emitted right before attention j, so the (readiness+priority)
    Tile scheduler uses proj matmuls as PE filler inside the
    ACT-limited attention stream — keeps the PE HAM-warm.
"""

import numpy as np
import ml_dtypes

import concourse.bass as bass
import concourse.tile as tile
from concourse import bacc, mybir
from concourse.bass import ts, ds

F32 = mybir.dt.float32
BF16 = mybir.dt.bfloat16
AF = mybir.ActivationFunctionType
ALU = mybir.AluOpType

B, T, C = 8, 2048, 384
H, DH = 6, 64
SCALE = DH ** -0.5
NCORES = 8
TJ = 512            # q-block width
NJ = T // TJ        # 4 q-blocks
SC = 128            # s-chunk
NCI = C // 128      # 3 channel chunks
NCH = TJ // SC      # fringe chunks per q-block (4)


def build_kernel():
    nc = bacc.Bacc("TRN2", target_bir_lowering=False, debug=False)

    xT_d = nc.dram_tensor("xT", [128, NCI, T], BF16, kind="ExternalInput").ap()
    wq_d = nc.dram_tensor("wq", [128, NCI, C], BF16, kind="ExternalInput").ap()
    wk_d = nc.dram_tensor("wk", [128, NCI, C], BF16, kind="ExternalInput").ap()
    wv_d = nc.dram_tensor("wv", [128, NCI, C], BF16, kind="ExternalInput").ap()
    wp_d = nc.dram_tensor("wp", [128, NCI, C], BF16, kind="ExternalInput").ap()
    biasb_d = nc.dram_tensor("biasb", [128, 384], F32, kind="ExternalInput").ap()
    y_d = nc.dram_tensor("y", [T, C], F32, kind="ExternalOutput").ap()

    with tile.TileContext(nc) as tc:
        with tc.tile_pool(name="const", bufs=1) as cpool, \
             tc.tile_pool(name="proj", bufs=1, space="PSUM") as projp, \
             tc.tile_pool(name="s2p", bufs=2, space="PSUM") as s2p, \
             tc.tile_pool(name="op", bufs=2, space="PSUM") as op, \
             tc.tile_pool(name="yp", bufs=1, space="PSUM") as yp, \
             tc.tile_pool(name="p2p", bufs=4) as p2p, \
             tc.tile_pool(name="r2p", bufs=2) as r2p, \
             tc.tile_pool(name="ysbp", bufs=2) as ysbp:
            xT = cpool.tile([128, NCI, T], BF16)
            wq = cpool.tile([128, NCI, C], BF16)
            wk = cpool.tile([128, NCI, C], BF16)
            wv = cpool.tile([128, NCI, C], BF16)
            wp = cpool.tile([128, NCI, C], BF16)
            biasb = cpool.tile([128, 384], F32)
            QT = cpool.tile([128, NCI, T], BF16)
            KT = cpool.tile([128, NCI, T], BF16)
            attT = cpool.tile([128, NCI, T], BF16)
            Vt = cpool.tile([128, 16, H, 65], BF16)

            for ci in range(NCI):
                nc.sync.dma_start(xT[:, ci, :], xT_d[:, ci, :])
            nc.sync.dma_start(wq[:], wq_d[:])
            nc.sync.dma_start(wk[:], wk_d[:])
            nc.sync.dma_start(wv[:], wv_d[:])
            nc.sync.dma_start(wp[:], wp_d[:])
            nc.sync.dma_start(biasb[:], biasb_d[:])
            # whole-tile memset (contiguous); V copies below overwrite
            # cols 0:64 of each [h, 65] group, leaving col 64 == 1.0
            nc.gpsimd.memset(Vt[:], 1.0)

            for j in range(NJ):
                # ---- projections for this q-block (PE filler work) ----
                for dst, w in ((KT, wk), (QT, wq)):
                    for pi in range(NCI):
                        ps = projp.tile([128, TJ], F32, tag="proj")
                        for ci in range(NCI):
                            nc.tensor.matmul(
                                ps[:],
                                lhsT=w[:, ci, ts(pi, 128)],
                                rhs=xT[:, ci, ts(j, TJ)],
                                start=(ci == 0), stop=(ci == NCI - 1),
                            )
                        nc.vector.tensor_copy(dst[:, pi, ts(j, TJ)], ps[:])
                for si in range(NCH * j, NCH * j + NCH):
                    ps = projp.tile([128, TJ], F32, tag="proj")
                    for ci in range(NCI):
                        nc.tensor.matmul(
                            ps[:, 0:C],
                            lhsT=xT[:, ci, ts(si, 128)],
                            rhs=wv[:, ci, :],
                            start=(ci == 0), stop=(ci == NCI - 1),
                        )
                    nc.vector.tensor_copy(
                        Vt[:, si, :, 0:64],
                        ps[:, 0:C].rearrange("p (h d) -> p h d", h=H),
                    )

                # ---- attention for q-block j, head pairs ----
                nch = NCH * j + NCH  # s-chunks for this q-block
                for bi in range(NCI):
                    h0, h1 = 2 * bi, 2 * bi + 1
                    O0 = op.tile([65, TJ], F32, tag="O")
                    O1 = op.tile([65, TJ], F32, tag="O")
                    for i in range(nch):
                        fringe = i >= NCH * j
                        d = SC * i - TJ * j if fringe else 0
                        S2 = s2p.tile([128, 2, TJ], F32, tag="S2")
                        # the two heads' S matmuls target different PE row
                        # groups (K=64 at partitions 0/64) → run concurrently
                        nc.tensor.matmul(
                            S2[:, 0, d:TJ],
                            lhsT=KT[0:64, bi, ts(i, SC)],
                            rhs=QT[0:64, bi, ds(j * TJ + d, TJ - d)],
                            start=True, stop=True,
                        )
                        nc.tensor.matmul(
                            S2[:, 1, d:TJ],
                            lhsT=KT[64:128, bi, ts(i, SC)],
                            rhs=QT[64:128, bi, ds(j * TJ + d, TJ - d)],
                            start=True, stop=True,
                        )
                        P2 = p2p.tile([128, 2, TJ], BF16, tag="P2")
                        nc.scalar.activation(P2[:, :, d:TJ], S2[:, :, d:TJ],
                                             AF.Exp, scale=SCALE)
                        if fringe:
                            # diagonal window [d, d+128): keep iff p <= f
                            for half in range(2):
                                nc.gpsimd.affine_select(
                                    out=P2[:, half, d:d + SC],
                                    in_=P2[:, half, d:d + SC],
                                    pattern=[[1, SC]],
                                    compare_op=ALU.is_ge,
                                    fill=0.0, base=0, channel_multiplier=-1,
                                )
                        nc.tensor.matmul(
                            O0[:, d:TJ],
                            lhsT=Vt[:, i, h0, :],
                            rhs=P2[:, 0, d:TJ],
                            start=(i == 0), stop=(i == nch - 1),
                        )
                        nc.tensor.matmul(
                            O1[:, d:TJ],
                            lhsT=Vt[:, i, h1, :],
                            rhs=P2[:, 1, d:TJ],
                            start=(i == 0), stop=(i == nch - 1),
                        )
    # normalization: copy denom rows to SBUF (gpsimd can't read
                    # PSUM), broadcast over partitions 0:64, reciprocal,
                    # multiply. All DVE input operands stay at partition
                    # base 0 (mismatched in0/in1 bases read wrong data);
                    # only outputs are partition-shifted.
                    dA = r2p.tile([1, TJ], F32, tag="dA")
                    dB = r2p.tile([1, TJ], F32, tag="dB")
                    nc.vector.tensor_copy(dA[:], O0[64:65, :])
                    nc.vector.tensor_copy(dB[:], O1[64:65, :])
                    RA = r2p.tile([64, TJ], F32, tag="RA")
                    RB = r2p.tile([64, TJ], F32, tag="RB")
                    nc.gpsimd.partition_broadcast(RA[:], dA[:])
                    nc.gpsimd.partition_broadcast(RB[:], dB[:])
                    nc.vector.reciprocal(RA[:], RA[:])
                    nc.vector.reciprocal(RB[:], RB[:])
                    nc.vector.tensor_tensor(
                        out=attT[0:64, bi, ts(j, TJ)], in0=O0[0:64, :],
                        in1=RA[:], op=ALU.mult,
                    )
                    nc.vector.tensor_tensor(
                        out=attT[64:128, bi, ts(j, TJ)], in0=O1[0:64, :],
                        in1=RB[:], op=ALU.mult,
                    )

                # ---- output projection for q-block j ----
                for q in range(NCH):
                    tb = NCH * j + q
                    Y = yp.tile([128, C], F32, tag="Y")
                    for bi in range(NCI):
                        nc.tensor.matmul(
                            Y[:],
                            lhsT=attT[:, bi, ts(tb, 128)],
                            rhs=wp[:, bi, :],
                            start=(bi == 0), stop=(bi == NCI - 1),
                        )
                    Ysb = ysbp.tile([128, C], F32, tag="Ysb")
                    nc.vector.scalar_tensor_tensor(
                        out=Ysb[:], in0=Y[:], scalar=1.0, in1=biasb[:],
                        op0=ALU.mult, op1=ALU.add,
                    )
                    nc.sync.dma_start(y_d[ts(tb, 128), :], Ysb[:])

    nc.compile()
    return nc


def _prep_inputs(x, Wq, Wk, Wv, Wp, bp):
    """Host-side shard + layout prep. Returns per-core input maps."""
    bf = ml_dtypes.bfloat16
    x = np.asarray(x, dtype=np.float32)

    def pack_w(W):  # [H, C, Dh] -> [128, NCI, H*Dh]
        Whd = np.transpose(np.asarray(W, np.float32), (1, 0, 2)).reshape(C, H * DH)
        return np.ascontiguousarray(
            Whd.reshape(NCI, 128, H * DH).transpose(1, 0, 2)
        ).astype(bf)

    wq_p, wk_p, wv_p = pack_w(Wq), pack_w(Wk), pack_w(Wv)
    wp_p = np.ascontiguousarray(
        np.asarray(Wp, np.float32).reshape(NCI, 128, C).transpose(1, 0, 2)
    ).astype(bf)

    biasb = np.broadcast_to(np.asarray(bp, np.float32), (128, C)).copy()

    in_maps = []
    for b in range(B):
        xT = np.ascontiguousarray(
            x[b].T.reshape(NCI, 128, T).transpose(1, 0, 2)
        ).astype(bf)
        in_maps.append({
            "xT": xT, "wq": wq_p, "wk": wk_p, "wv": wv_p, "wp": wp_p,
            "biasb": biasb,
        })
    return in_maps


_CACHE = {}


def kernel(x, Wq, Wk, Wv, Wp, bp):
    from concourse.bass_utils import run_bass_kernel_spmd

    if "nc" not in _CACHE:
        _CACHE["nc"] = build_kernel()
    nc = _CACHE["nc"]
    in_maps = _prep_inputs(x, Wq, Wk, Wv, Wp, bp)
    res = run_bass_kernel_spmd(nc, in_maps, list(range(NCORES)))
    out = np.stack([res.results[b]["y"] for b in range(B)], axis=0)
    return out.astype(np.float32)


# revision 7
# speedup vs baseline: 1.1753x; 1.0745x over previous
"""Multi-head causal attention (B=8, T=2048, C=384, H=6, Dh=64) on 8 TRN2 cores.

Sharding: data-parallel over batch — core b computes batch element b end to end
(no collectives).

v2 design notes (vs v1):
  - Head-PAIR processing: heads (2bi, 2bi+1) live on partition halves
    [0:64) / [64:128) of hd-block bi. Their S matmuls (K=64) are issued
    back-to-back so the PE runs them CONCURRENTLY via row tiling
    (tile_position auto-derived from base_partition 0 / 64).
  - One wide ACTIVATE (exp) per chunk-pair over S2 [128, 2, 512-d]
    (both heads at once) — halves ACT instruction overhead.
  - PV fringe-trimmed: matmul only cols [d:512] (P[:, :d] is never read
    → no memsets). Softmax denominator via V_aug ones-row (M=65).
  - Normalization: denominator rows broadcast across partitions on
    GpSimd (partition_broadcast), reciprocal on DVE, then one
    tensor_tensor multiply straight out of PSUM O → attT (bf16).
    No PE transposes, no per-head STT chains.
  - Output projection packed at K=128 (3 matmuls per 128-token block),
    bias added by the PSUM→SBUF scalar_tensor_tensor copy.
  - QKV projections are NOT a separate phase: proj for q-block j is
    emitted right before attention j, so the (readiness+priority)
    Tile scheduler uses proj matmuls as PE filler inside the
    ACT-limited attention stream — keeps the PE HAM-warm.
"""

import numpy as np
import ml_dtypes

import concourse.bass as bass
import concourse.tile as tile
from concourse import bacc, mybir
from concourse.bass import ts, ds

F32 = mybir.dt.float32
BF16 = mybir.dt.bfloat16
AF = mybir.ActivationFunctionType
ALU = mybir.AluOpType

B, T, C = 8, 2048, 384
H, DH = 6, 64
SCALE = DH ** -0.5
NCORES = 8
TJ = 512            # q-block width
NJ = T // TJ        # 4 q-blocks
SC = 128            # s-chunk
NCI = C // 128      # 3 channel chunks
NCH = TJ // SC      # fringe chunks per q-block (4)


def build_kernel():
    nc = bacc.Bacc("TRN2", target_bir_lowering=False, debug=False)

    xT_d = nc.dram_tensor("xT", [128, NCI, T], BF16, kind="ExternalInput").ap()
    wq_d = nc.dram_tensor("wq", [128, NCI, C], BF16, kind="ExternalInput").ap()
    wk_d = nc.dram_tensor("wk", [128, NCI, C], BF16, kind="ExternalInput").ap()
    wv_d = nc.dram_tensor("wv", [128, NCI, C], BF16, kind="ExternalInput").ap()
    wp_d = nc.dram_tensor("wp", [128, NCI, C], BF16, kind="ExternalInput").ap()
    biasb_d = nc.dram_tensor("biasb", [128, 384], F32, kind="ExternalInput").ap()
    y_d = nc.dram_tensor("y", [T, C], F32, kind="ExternalOutput").ap()

    with tile.TileContext(nc) as tc:
        with tc.tile_pool(name="const", bufs=1) as cpool, \
             tc.tile_pool(name="proj", bufs=1, space="PSUM") as projp, \
             tc.tile_pool(name="s2p", bufs=2, space="PSUM") as s2p, \
             tc.tile_pool(name="op", bufs=3, space="PSUM") as op, \
             tc.tile_pool(name="p2p", bufs=4) as p2p, \
             tc.tile_pool(name="r2p", bufs=2) as r2p, \
             tc.tile_pool(name="ysbp", bufs=2) as ysbp:
            xT = cpool.tile([128, NCI, T], BF16)
            wq = cpool.tile([128, NCI, C], BF16)
            wk = cpool.tile([128, NCI, C], BF16)
            wv = cpool.tile([128, NCI, C], BF16)
            wp = cpool.tile([128, NCI, C], BF16)
            biasb = cpool.tile([128, 384], F32)
            QT = cpool.tile([128, NCI, T], BF16)
            KT = cpool.tile([128, NCI, T], BF16)
            attT = cpool.tile([128, NCI, T], BF16)
            Vt = cpool.tile([128, 16, H, 65], BF16)

            for ci in range(NCI):
                nc.sync.dma_start(xT[:, ci, :], xT_d[:, ci, :])
            nc.sync.dma_start(wq[:], wq_d[:])
            nc.sync.dma_start(wk[:], wk_d[:])
            nc.sync.dma_start(wv[:], wv_d[:])
            nc.sync.dma_start(wp[:], wp_d[:])
            nc.sync.dma_start(biasb[:], biasb_d[:])
            # whole-tile memset (contiguous); V copies below overwrite
            # cols 0:64 of each [h, 65] group, leaving col 64 == 1.0
            nc.gpsimd.memset(Vt[:], 1.0)

            for j in range(NJ):
                # ---- projections for this q-block (PE filler work) ----
                for dst, w in ((KT, wk), (QT, wq)):
                    for pi in range(NCI):
                        ps = projp.tile([128, TJ], F32, tag="proj")
                        for ci in range(NCI):
                            nc.tensor.matmul(
                                ps[:],
                                lhsT=w[:, ci, ts(pi, 128)],
                                rhs=xT[:, ci, ts(j, TJ)],
                                start=(ci == 0), stop=(ci == NCI - 1),
                            )
                        nc.vector.tensor_copy(dst[:, pi, ts(j, TJ)], ps[:])
                for si in range(NCH * j, NCH * j + NCH):
                    ps = projp.tile([128, TJ], F32, tag="proj")
                    for ci in range(NCI):
                        nc.tensor.matmul(
                            ps[:, 0:C],
                            lhsT=xT[:, ci, ts(si, 128)],
                            rhs=wv[:, ci, :],
                            start=(ci == 0), stop=(ci == NCI - 1),
                        )
                    nc.vector.tensor_copy(
                        Vt[:, si, :, 0:64],
                        ps[:, 0:C].rearrange("p (h d) -> p h d", h=H),
                    )

                # ---- attention for q-block j, head pairs ----
                nch = NCH * j + NCH  # s-chunks for this q-block
                for bi in range(NCI):
                    h0, h1 = 2 * bi, 2 * bi + 1
                    O0 = op.tile([65, TJ], F32, tag="O")
                    O1 = op.tile([65, TJ], F32, tag="O")
                    for i in range(nch):
                        fringe = i >= NCH * j
                        d = SC * i - TJ * j if fringe else 0
                        S2 = s2p.tile([128, 2, TJ], F32, tag="S2")
                        # the two heads' S matmuls target different PE row
                        # groups (K=64 at partitions 0/64) → run concurrently
                        nc.tensor.matmul(
                            S2[:, 0, d:TJ],
                            lhsT=KT[0:64, bi, ts(i, SC)],
                            rhs=QT[0:64, bi, ds(j * TJ + d, TJ - d)],
                            start=True, stop=True,
                        )
                        nc.tensor.matmul(
                            S2[:, 1, d:TJ],
                            lhsT=KT[64:128, bi, ts(i, SC)],
                            rhs=QT[64:128, bi, ds(j * TJ + d, TJ - d)],
                            start=True, stop=True,
                        )
                        P2 = p2p.tile([128, 2, TJ], BF16, tag="P2")
                        nc.scalar.activation(P2[:, :, d:TJ], S2[:, :, d:TJ],
                                             AF.Exp, scale=SCALE)
                        if fringe:
                            # diagonal window [d, d+128): keep iff p <= f
                            for half in range(2):
                                nc.gpsimd.affine_select(
                                    out=P2[:, half, d:d + SC],
                                    in_=P2[:, half, d:d + SC],
                                    pattern=[[1, SC]],
                                    compare_op=ALU.is_ge,
                                    fill=0.0, base=0, channel_multiplier=-1,
                                )
                        nc.tensor.matmul(
                            O0[:, d:TJ],
                            lhsT=Vt[:, i, h0, :],
                            rhs=P2[:, 0, d:TJ],
                            start=(i == 0), stop=(i == nch - 1),
                        )
                        nc.tensor.matmul(
                            O1[:, d:TJ],
                            lhsT=Vt[:, i, h1, :],
                            rhs=P2[:, 1, d:TJ],
                            start=(i == 0), stop=(i == nch - 1),
                        )
    # normalization: reciprocal of the denom rows straight out of
                    # PSUM (approx_fast: ~18 bits, denominators are >= 1),
                    # broadcast over partitions 0:64 on gpsimd, multiply.
                    # All DVE input operands stay at partition base 0
                    # (mismatched in0/in1 bases read wrong data); only
                    # outputs are partition-shifted.
                    dA = r2p.tile([1, TJ], F32, tag="dA")
                    dB = r2p.tile([1, TJ], F32, tag="dB")
                    rA = r2p.tile([1, TJ], F32, tag="rA")
                    rB = r2p.tile([1, TJ], F32, tag="rB")
                    nc.vector.tensor_copy(dA[:], O0[64:65, :])
                    nc.vector.tensor_copy(dB[:], O1[64:65, :])
                    nc.vector.reciprocal_approx_fast(rA[:], dA[:])
                    nc.vector.reciprocal_approx_fast(rB[:], dB[:])
                    RA = r2p.tile([64, TJ], F32, tag="RA")
                    RB = r2p.tile([64, TJ], F32, tag="RB")
                    nc.gpsimd.partition_broadcast(RA[:], rA[:])
                    nc.gpsimd.partition_broadcast(RB[:], rB[:])
                    nc.vector.tensor_tensor(
                        out=attT[0:64, bi, ts(j, TJ)], in0=O0[0:64, :],
                        in1=RA[:], op=ALU.mult,
                    )
                    nc.vector.tensor_tensor(
                        out=attT[64:128, bi, ts(j, TJ)], in0=O1[0:64, :],
                        in1=RB[:], op=ALU.mult,
                    )

                # ---- output projection for q-block j ----
                for q in range(NCH):
                    tb = NCH * j + q
                    Y = projp.tile([128, TJ], F32, tag="proj", name="Y")[:, 0:C]
                    for bi in range(NCI):
                        nc.tensor.matmul(
                            Y[:],
                            lhsT=attT[:, bi, ts(tb, 128)],
                            rhs=wp[:, bi, :],
                            start=(bi == 0), stop=(bi == NCI - 1),
                        )
                    Ysb = ysbp.tile([128, C], F32, tag="Ysb")
                    nc.vector.scalar_tensor_tensor(
                        out=Ysb[:], in0=Y[:], scalar=1.0, in1=biasb[:],
                        op0=ALU.mult, op1=ALU.add,
                    )
                    nc.sync.dma_start(y_d[ts(tb, 128), :], Ysb[:])

    nc.compile()
    return nc


def _prep_inputs(x, Wq, Wk, Wv, Wp, bp):
    """Host-side shard + layout prep. Returns per-core input maps."""
    bf = ml_dtypes.bfloat16
    x = np.asarray(x, dtype=np.float32)

    def pack_w(W):  # [H, C, Dh] -> [128, NCI, H*Dh]
        Whd = np.transpose(np.asarray(W, np.float32), (1, 0, 2)).reshape(C, H * DH)
        return np.ascontiguousarray(
            Whd.reshape(NCI, 128, H * DH).transpose(1, 0, 2)
        ).astype(bf)

    wq_p, wk_p, wv_p = pack_w(Wq), pack_w(Wk), pack_w(Wv)
    wp_p = np.ascontiguousarray(
        np.asarray(Wp, np.float32).reshape(NCI, 128, C).transpose(1, 0, 2)
    ).astype(bf)

    biasb = np.broadcast_to(np.asarray(bp, np.float32), (128, C)).copy()

    in_maps = []
    for b in range(B):
        xT = np.ascontiguousarray(
            x[b].T.reshape(NCI, 128, T).transpose(1, 0, 2)
        ).astype(bf)
        in_maps.append({
            "xT": xT, "wq": wq_p, "wk": wk_p, "wv": wv_p, "wp": wp_p,
            "biasb": biasb,
        })
    return in_maps


_CACHE = {}


def kernel(x, Wq, Wk, Wv, Wp, bp):
    from concourse.bass_utils import run_bass_kernel_spmd

    if "nc" not in _CACHE:
        _CACHE["nc"] = build_kernel()
    nc = _CACHE["nc"]
    in_maps = _prep_inputs(x, Wq, Wk, Wv, Wp, bp)
    res = run_bass_kernel_spmd(nc, in_maps, list(range(NCORES)))
    out = np.stack([res.results[b]["y"] for b in range(B)], axis=0)
    return out.astype(np.float32)


# revision 9
# speedup vs baseline: 1.5352x; 1.3061x over previous
"""Multi-head causal attention (B=8, T=2048, C=384, H=6, Dh=64) on 8 TRN2 cores.

Sharding: data-parallel over batch — core b computes batch element b end to end
(no collectives).

v2 design notes (vs v1):
  - Head-PAIR processing: heads (2bi, 2bi+1) live on partition halves
    [0:64) / [64:128) of hd-block bi. Their S matmuls (K=64) are issued
    back-to-back so the PE runs them CONCURRENTLY via row tiling
    (tile_position auto-derived from base_partition 0 / 64).
  - One wide ACTIVATE (exp) per chunk-pair over S2 [128, 2, 512-d]
    (both heads at once) — halves ACT instruction overhead.
  - PV fringe-trimmed: matmul only cols [d:512] (P[:, :d] is never read
    → no memsets). Softmax denominator via V_aug ones-row (M=65).
  - Normalization: denominator rows broadcast across partitions on
    GpSimd (partition_broadcast), reciprocal on DVE, then one
    tensor_tensor multiply straight out of PSUM O → attT (bf16).
    No PE transposes, no per-head STT chains.
  - Output projection packed at K=128 (3 matmuls per 128-token block),
    bias added by the PSUM→SBUF scalar_tensor_tensor copy.
  - QKV projections are NOT a separate phase: proj for q-block j is
    emitted right before attention j, so the (readiness+priority)
    Tile scheduler uses proj matmuls as PE filler inside the
    ACT-limited attention stream — keeps the PE HAM-warm.
"""

import numpy as np
import ml_dtypes

import concourse.bass as bass
import concourse.tile as tile
from concourse import bacc, mybir
from concourse.bass import ts, ds

F32 = mybir.dt.float32
BF16 = mybir.dt.bfloat16
AF = mybir.ActivationFunctionType
ALU = mybir.AluOpType

B, T, C = 8, 2048, 384
H, DH = 6, 64
SCALE = DH ** -0.5
NCORES = 8
TJ = 512            # q-block width
NJ = T // TJ        # 4 q-blocks
SC = 128            # s-chunk
NCI = C // 128      # 3 channel chunks
NCH = TJ // SC      # fringe chunks per q-block (4)


def build_kernel():
    nc = bacc.Bacc("TRN2", target_bir_lowering=False, debug=False)

    xT_d = nc.dram_tensor("xT", [128, NCI, T], BF16, kind="ExternalInput").ap()
    wq_d = nc.dram_tensor("wq", [128, NCI, C], BF16, kind="ExternalInput").ap()
    wk_d = nc.dram_tensor("wk", [128, NCI, C], BF16, kind="ExternalInput").ap()
    wv_d = nc.dram_tensor("wv", [128, NCI, C], BF16, kind="ExternalInput").ap()
    wp_d = nc.dram_tensor("wp", [128, NCI, C], BF16, kind="ExternalInput").ap()
    biasb_d = nc.dram_tensor("biasb", [128, 384], F32, kind="ExternalInput").ap()
    y_d = nc.dram_tensor("y", [T, C], F32, kind="ExternalOutput").ap()

    with tile.TileContext(nc) as tc:
        with tc.tile_pool(name="const", bufs=1) as cpool, \
             tc.tile_pool(name="proj", bufs=1, space="PSUM") as projp, \
             tc.tile_pool(name="s2p", bufs=2, space="PSUM") as s2p, \
             tc.tile_pool(name="op", bufs=3, space="PSUM") as op, \
             tc.tile_pool(name="p2p", bufs=4) as p2p, \
             tc.tile_pool(name="r2p", bufs=2) as r2p, \
             tc.tile_pool(name="ysbp", bufs=2) as ysbp:
            xT = cpool.tile([128, NCI, T], BF16)
            wq = cpool.tile([128, NCI, C], BF16)
            wk = cpool.tile([128, NCI, C], BF16)
            wv = cpool.tile([128, NCI, C], BF16)
            wp = cpool.tile([128, NCI, C], BF16)
            biasb = cpool.tile([128, 384], F32)
            QT = cpool.tile([128, NCI, T], BF16)
            KT = cpool.tile([128, NCI, T], BF16)
            attT = cpool.tile([128, NCI, T], BF16)
            Vt = cpool.tile([128, 16, H, 65], BF16)

            nc.sync.dma_start(wq[:], wq_d[:])
            nc.sync.dma_start(wk[:], wk_d[:])
            nc.sync.dma_start(wv[:], wv_d[:])
            nc.sync.dma_start(wp[:], wp_d[:])
            nc.sync.dma_start(biasb[:], biasb_d[:])
            for tcn in range(NJ):
                for ci in range(NCI):
                    nc.sync.dma_start(xT[:, ci, ts(tcn, TJ)],
                                      xT_d[:, ci, ts(tcn, TJ)])
            # whole-tile memset (contiguous); V copies below overwrite
            # cols 0:64 of each [h, 65] group, leaving col 64 == 1.0
            nc.gpsimd.memset(Vt[:], 1.0)

            for j in range(NJ):
                # ---- projections for this q-block (PE filler work) ----
                for dst, w in ((KT, wk), (QT, wq)):
                    for pi in range(NCI):
                        ps = projp.tile([128, TJ], F32, tag="proj")
                        for ci in range(NCI):
                            nc.tensor.matmul(
                                ps[:],
                                lhsT=w[:, ci, ts(pi, 128)],
                                rhs=xT[:, ci, ts(j, TJ)],
                                start=(ci == 0), stop=(ci == NCI - 1),
                            )
                        nc.vector.tensor_copy(dst[:, pi, ts(j, TJ)], ps[:])
                for si in range(NCH * j, NCH * j + NCH):
                    ps = projp.tile([128, TJ], F32, tag="proj")
                    for ci in range(NCI):
                        nc.tensor.matmul(
                            ps[:, 0:C],
                            lhsT=xT[:, ci, ts(si, 128)],
                            rhs=wv[:, ci, :],
                            start=(ci == 0), stop=(ci == NCI - 1),
                        )
                    nc.vector.tensor_copy(
                        Vt[:, si, :, 0:64],
                        ps[:, 0:C].rearrange("p (h d) -> p h d", h=H),
                    )

                # ---- attention for q-block j, head pairs ----
                nch = NCH * j + NCH  # s-chunks for this q-block
                for bi in range(NCI):
                    h0, h1 = 2 * bi, 2 * bi + 1
                    O0 = op.tile([65, TJ], F32, tag="O")
                    O1 = op.tile([65, TJ], F32, tag="O")
                    for i in range(nch):
                        fringe = i >= NCH * j
                        d = SC * i - TJ * j if fringe else 0
                        S2 = s2p.tile([128, 2, TJ], F32, tag="S2")
                        # the two heads' S matmuls target different PE row
                        # groups (K=64 at partitions 0/64) → run concurrently
                        nc.tensor.matmul(
                            S2[:, 0, d:TJ],
                            lhsT=KT[0:64, bi, ts(i, SC)],
                            rhs=QT[0:64, bi, ds(j * TJ + d, TJ - d)],
                            start=True, stop=True,
                        )
                        nc.tensor.matmul(
                            S2[:, 1, d:TJ],
                            lhsT=KT[64:128, bi, ts(i, SC)],
                            rhs=QT[64:128, bi, ds(j * TJ + d, TJ - d)],
                            start=True, stop=True,
                        )
                        P2 = p2p.tile([128, 2, TJ], BF16, tag="P2")
                        nc.scalar.activation(P2[:, :, d:TJ], S2[:, :, d:TJ],
                                             AF.Exp, scale=SCALE)
                        if fringe:
                            # diagonal window [d, d+128): keep iff p <= f
                            for half in range(2):
                                nc.gpsimd.affine_select(
                                    out=P2[:, half, d:d + SC],
                                    in_=P2[:, half, d:d + SC],
                                    pattern=[[1, SC]],
                                    compare_op=ALU.is_ge,
                                    fill=0.0, base=0, channel_multiplier=-1,
                                )
                        nc.tensor.matmul(
                            O0[:, d:TJ],
                            lhsT=Vt[:, i, h0, :],
                            rhs=P2[:, 0, d:TJ],
                            start=(i == 0), stop=(i == nch - 1),
                        )
                        nc.tensor.matmul(
                            O1[:, d:TJ],
                            lhsT=Vt[:, i, h1, :],
                            rhs=P2[:, 1, d:TJ],
                            start=(i == 0), stop=(i == nch - 1),
                        )
    # normalization: reciprocal of the denom rows straight out of
                    # PSUM (approx_fast: ~18 bits, denominators are >= 1),
                    # broadcast over partitions 0:64 on gpsimd, multiply.
                    # All DVE input operands stay at partition base 0
                    # (mismatched in0/in1 bases read wrong data); only
                    # outputs are partition-shifted.
                    dA = r2p.tile([1, TJ], F32, tag="dA")
                    dB = r2p.tile([1, TJ], F32, tag="dB")
                    rA = r2p.tile([1, TJ], F32, tag="rA")
                    rB = r2p.tile([1, TJ], F32, tag="rB")
                    nc.vector.tensor_copy(dA[:], O0[64:65, :])
                    nc.vector.tensor_copy(dB[:], O1[64:65, :])
                    nc.vector.reciprocal_approx_fast(rA[:], dA[:])
                    nc.vector.reciprocal_approx_fast(rB[:], dB[:])
                    RA = r2p.tile([64, TJ], F32, tag="RA")
                    RB = r2p.tile([64, TJ], F32, tag="RB")
                    nc.gpsimd.partition_broadcast(RA[:], rA[:])
                    nc.gpsimd.partition_broadcast(RB[:], rB[:])
                    nc.vector.tensor_tensor(
                        out=attT[0:64, bi, ts(j, TJ)], in0=O0[0:64, :],
                        in1=RA[:], op=ALU.mult,
                    )
                    nc.vector.tensor_tensor(
                        out=attT[64:128, bi, ts(j, TJ)], in0=O1[0:64, :],
                        in1=RB[:], op=ALU.mult,
                    )

                # ---- output projection for q-block j ----
                for q in range(NCH):
                    tb = NCH * j + q
                    Y = op.tile([128, C], F32, tag="O", name="Y")
                    for bi in range(NCI):
                        nc.tensor.matmul(
                            Y[:],
                            lhsT=attT[:, bi, ts(tb, 128)],
                            rhs=wp[:, bi, :],
                            start=(bi == 0), stop=(bi == NCI - 1),
                        )
                    Ysb = ysbp.tile([128, C], F32, tag="Ysb")
                    nc.vector.scalar_tensor_tensor(
                        out=Ysb[:], in0=Y[:], scalar=1.0, in1=biasb[:],
                        op0=ALU.mult, op1=ALU.add,
                    )
                    nc.sync.dma_start(y_d[ts(tb, 128), :], Ysb[:])

    nc.compile()
    return nc


def _prep_inputs(x, Wq, Wk, Wv, Wp, bp):
    """Host-side shard + layout prep. Returns per-core input maps."""
    bf = ml_dtypes.bfloat16
    x = np.asarray(x, dtype=np.float32)

    def pack_w(W):  # [H, C, Dh] -> [128, NCI, H*Dh]
        Whd = np.transpose(np.asarray(W, np.float32), (1, 0, 2)).reshape(C, H * DH)
        return np.ascontiguousarray(
            Whd.reshape(NCI, 128, H * DH).transpose(1, 0, 2)
        ).astype(bf)

    wq_p, wk_p, wv_p = pack_w(Wq), pack_w(Wk), pack_w(Wv)
    wp_p = np.ascontiguousarray(
        np.asarray(Wp, np.float32).reshape(NCI, 128, C).transpose(1, 0, 2)
    ).astype(bf)

    biasb = np.broadcast_to(np.asarray(bp, np.float32), (128, C)).copy()

    in_maps = []
    for b in range(B):
        xT = np.ascontiguousarray(
            x[b].T.reshape(NCI, 128, T).transpose(1, 0, 2)
        ).astype(bf)
        in_maps.append({
            "xT": xT, "wq": wq_p, "wk": wk_p, "wv": wv_p, "wp": wp_p,
            "biasb": biasb,
        })
    return in_maps


_CACHE = {}


def kernel(x, Wq, Wk, Wv, Wp, bp):
    from concourse.bass_utils import run_bass_kernel_spmd

    if "nc" not in _CACHE:
        _CACHE["nc"] = build_kernel()
    nc = _CACHE["nc"]
    in_maps = _prep_inputs(x, Wq, Wk, Wv, Wp, bp)
    res = run_bass_kernel_spmd(nc, in_maps, list(range(NCORES)))
    out = np.stack([res.results[b]["y"] for b in range(B)], axis=0)
    return out.astype(np.float32)
